# revision 1
# baseline (speedup 1.0000x reference)
"""ChebConv-style GNN message passing kernel for Trainium2 (8 NeuronCores).

Strategy (see spec sharding_hint): destination-node graph partition across the
8 cores. Each core owns N/8 destination nodes. Per Chebyshev step k:
  - gather source-node feature rows t[col[e]] (node-major [N, Q*H*F] layout,
    one contiguous fp16 row per node) from a full replica in the core's HBM
    via GPSIMD dma_gather (<=1024 idx per call),
  - segment-sum + edge scaling via PE matmuls: for each 128-edge chunk a
    stationary matrix S'[e, r] = -w_e * dis[col_e] placed at r = row_e mod 128
    (one-hot by destination) contracts the 128 gathered messages into the
    128-node destination tile, accumulating across chunks in fp32 PSUM,
  - Tx_k tile = 2 * dis_row * PSUM - Tx_{k-2} (Chebyshev recurrence) on
    ACT/DVE, stored fp16,
  - AllGather the per-core Tx_k slices into a full replica for the next step.
Final einsum sum_k Tx_k[n, q, :] @ W_k runs via HWDGE DMA-transpose loads +
PE matmuls with fp32 PSUM accumulation over k, plus bias.

Host-side preprocessing is limited to graph partitioning / index plumbing
(sorting edges by destination tile, degree counts, building the one-hot
scatter matrices and int16 gather-index lists) and data layout (transposes,
fp16 casts). All O(E*Q*H*F) tensor math runs on device.
"""
import numpy as np
from contextlib import ExitStack

Q, N, H, F, G, K, E = 2, 50000, 4, 32, 32, 4, 800000
NCORES = 8
TILES = 49                 # 128-node destination tiles per core
NPC = TILES * 128          # 6272 owned (padded) nodes per core
NPAD = NCORES * NPC        # 50176
D = Q * H * F              # 256 features per node (node-major packing)
SPLIT = 32768              # int16 gather-index limit -> two source segments
GMAX = 1024                # dma_gather hangs above 1024 idxs per call
FP16 = True                # fp16 data path (fp32 PSUM accumulation)

_cache = {}
TRACE = False
LAST_EXEC_NS = None
LAST_RESULT = None


def configure(n, tiles, split, fp16=None):
    """Shrink the problem (debug only)."""
    global N, TILES, NPC, NPAD, SPLIT, FP16
    N, TILES, SPLIT = n, tiles, split
    NPC = TILES * 128
    NPAD = NCORES * NPC
    if fp16 is not None:
        FP16 = fp16
    assert N <= NPAD


def _build_program(T0, T1, repeats=1, skip=()):
    import concourse.bacc as bacc
    import concourse.mybir as mybir
    import concourse.tile as tile
    from concourse.library_config import mlp

    NCH0, NCH1 = T0 // 128, T1 // 128
    NCH = NCH0 + NCH1
    T = T0 + T1

    nc = bacc.Bacc("TRN2", target_bir_lowering=False, debug=False,
                   num_devices=NCORES, num_swdge_queues=4,
                   dynamic_dma_scratch_size=65536)
    f32 = mybir.dt.float32
    dt = mybir.dt.float16 if FP16 else f32
    xin = nc.dram_tensor("xin", [NPAD, D], dt, kind="ExternalInput")
    sp = nc.dram_tensor("sp", [TILES, 128, NCH * 128], dt, kind="ExternalInput")
    idx = nc.dram_tensor("idx", [TILES, 128, T // 16], mybir.dt.int16,
                         kind="ExternalInput")
    discol = nc.dram_tensor("discol", [128, 2 * TILES], f32, kind="ExternalInput")
    wts = nc.dram_tensor("wts", [128, K * G], dt, kind="ExternalInput")
    biasrep = nc.dram_tensor("biasrep", [128, G], f32, kind="ExternalInput")
    ident = nc.dram_tensor("ident", [128, 128], f32, kind="ExternalInput")
    xown = nc.dram_tensor("xown", [NPC, D], dt, kind="ExternalInput")
    out = nc.dram_tensor("out", [Q, NPC, G], f32, kind="ExternalOutput")

    with tile.TileContext(nc) as tc, ExitStack() as ctx:
        nc.gpsimd.load_library(mlp)
        dram = ctx.enter_context(tc.tile_pool(name="dram", bufs=1, space="DRAM"))
        const = ctx.enter_context(tc.tile_pool(name="const", bufs=1))
        sbuf = ctx.enter_context(tc.tile_pool(name="sbuf", bufs=3))
        spool = ctx.enter_context(tc.tile_pool(name="spool", bufs=3))
        txp = ctx.enter_context(tc.tile_pool(name="txp", bufs=4))
        psum = ctx.enter_context(tc.tile_pool(name="psum", bufs=2, space="PSUM"))
        ptr = ctx.enter_context(tc.tile_pool(name="ptr", bufs=2, space="PSUM"))

        # own Tx slices (k = 1..3) + allgathered replicas (k = 1, 2)
        txown = [None] + [dram.tile([NPC, D], dt, name=f"txown{i}",
                                    tag=f"txown{i}") for i in range(3)]

        # constants
        disc = const.tile([128, 2 * TILES], f32)
        nc.sync.dma_start(disc[:], discol.ap())
        wt = const.tile([128, K * G], dt)
        nc.sync.dma_start(wt[:], wts.ap())
        bt = const.tile([128, G], f32)
        nc.sync.dma_start(bt[:], biasrep.ap())
        idt = const.tile([128, 128], f32)
        nc.sync.dma_start(idt[:], ident.ap())

        xin_ap = xin.ap()
        gq = [0]  # round-robin dma_gather queue across the 4 Q7 core pairs

        for _rep in range(repeats):
            txr = [None] + [dram.tile([NPAD, D], dt, name=f"txr{i}_{_rep}",
                                      tag=f"txr{i}_{_rep}", addr_space="Shared")
                            for i in range(2)] + [None]
            for k in (1, 2, 3):
                if k == 1:
                    rep = xin_ap
                else:
                    rep = txr[k - 1][:]
                src_low, src_high = rep[0:SPLIT, :], rep[SPLIT:NPAD, :]
                if k == 2:
                    prevsrc = xown.ap()    # Tx_0 own slice
                elif k == 3:
                    prevsrc = txown[1][:]  # Tx_1 own slice
                for t in range(TILES):
                    idx_t = sbuf.tile([128, T // 16], mybir.dt.int16, tag="idx")
                    nc.sync.dma_start(idx_t[:], idx.ap()[t])
                    sp_t = spool.tile([128, NCH * 128], dt, tag="sp")
                    if "sp" not in skip:
                        nc.sync.dma_start(sp_t[:], sp.ap()[t])
                    msg = sbuf.tile([128, NCH, D], dt, tag="msg")
                    for src_seg, s0, ns in (() if "gath" in skip else
                                            ((src_low, 0, T0), (src_high, T0, T1))):
                        for a in range(0, ns, GMAX):
                            L = min(GMAX, ns - a)
                            sb = s0 + a
                            nc.gpsimd.dma_gather(
                                msg[:, sb // 128:(sb + L) // 128, :], src_seg,
                                idx_t[:, sb // 16:(sb + L) // 16], L, L, D,
                                queue_num=gq[0])
                            gq[0] = (gq[0] + 1) % 4
                    acc = psum.tile([128, D], f32, tag="acc")
                    for c in ([] if "mm" in skip else range(NCH)):
                        nc.tensor.matmul(acc[:], sp_t[:, c * 128:(c + 1) * 128],
                                         msg[:, c, :], start=(c == 0),
                                         stop=(c == NCH - 1))
                    txt = txp.tile([128, D], dt, tag="txt")
                    if k == 1:
                        # Tx_1 = dis_r * acc
                        nc.scalar.activation(txt[:], acc[:],
                                             mybir.ActivationFunctionType.Copy,
                                             scale=disc[:, t:t + 1])
                    else:
                        # Tx_k = 2 * dis_r * acc - Tx_{k-2}
                        prev = txp.tile([128, D], dt, tag="prev")
                        nc.sync.dma_start(prev[:],
                                          prevsrc[t * 128:(t + 1) * 128, :])
                        tmp = txp.tile([128, D], f32, tag="tmp")
                        nc.scalar.activation(tmp[:], acc[:],
                                             mybir.ActivationFunctionType.Copy,
                                             scale=disc[:, TILES + t:TILES + t + 1])
                        nc.vector.tensor_tensor(txt[:], tmp[:], prev[:],
                                                mybir.AluOpType.subtract)
                    nc.sync.dma_start(txown[k][:][t * 128:(t + 1) * 128, :],
                                      txt[:])
                if k < 3 and "coll" not in skip:
                    nc.gpsimd.collective_compute(
                        "AllGather", mybir.AluOpType.bypass,
                        replica_groups=[list(range(NCORES))],
                        ins=[txown[k].opt()], outs=[txr[k].opt()])

            # ---- einsum: out[q, n, g] = sum_k Tx_k[n, q, :] @ W_k + bias
            for t in ([] if "eins" in skip else range(TILES)):
                for q in range(Q):
                    pacc = ptr.tile([128, G], f32, tag="pacc")
                    for k in range(K):
                        src = xown.ap() if k == 0 else txown[k][:]
                        reg = src[t * 128:(t + 1) * 128, q * 128:(q + 1) * 128]
                        at = sbuf.tile([128, 128], dt, tag="eins_at")
                        if FP16:
                            nc.sync.dma_start(at[:], reg, transpose=True)
                        else:
                            a = sbuf.tile([128, 128], f32, tag="eins_a")
                            nc.sync.dma_start(a[:], reg)
                            pt = ptr.tile([128, 128], f32, tag="pt")
                            nc.tensor.transpose(pt[:], a[:], idt[:])
                            nc.scalar.activation(
                                at[:], pt[:], mybir.ActivationFunctionType.Copy)
                        nc.tensor.matmul(pacc[:], at[:],
                                         wt[:, k * G:(k + 1) * G],
                                         start=(k == 0), stop=(k == K - 1),
                                         skip_group_check=True)
                    ot = txp.tile([128, G], f32, tag="ot")
                    nc.vector.tensor_tensor(ot[:], pacc[:], bt[:],
                                            mybir.AluOpType.add)
                    nc.sync.dma_start(out.ap()[q, t * 128:(t + 1) * 128, :],
                                      ot[:])

    nc.compile()
    return nc


def _preprocess(x, edge_index, edge_weight):
    """Graph partitioning + index/layout prep (host)."""
    row = np.asarray(edge_index[0], dtype=np.int64)
    col = np.asarray(edge_index[1], dtype=np.int64)
    w = np.asarray(edge_weight, dtype=np.float32)
    mask = row != col
    deg = np.bincount(row[mask], minlength=N).astype(np.float32)
    dis = np.where(deg > 0, deg ** -0.5, 0.0).astype(np.float32)

    keep = mask & (dis[col] > 0)
    r2, c2, w2 = row[keep], col[keep], w[keep]
    val = (-w2 * dis[c2]).astype(np.float32)

    tile_g = r2 // 128                      # global tile id
    rel = (r2 % 128).astype(np.int64)
    is_high = (c2 >= SPLIT).astype(np.int64)

    key = tile_g * 2 + is_high
    order = np.argsort(key, kind="stable")
    key_s = key[order]
    ngroups = NCORES * TILES * 2
    starts = np.searchsorted(key_s, np.arange(ngroups))
    ends = np.searchsorted(key_s, np.arange(ngroups) + 1)
    counts = ends - starts
    cnt_low = counts[0::2]
    cnt_high = counts[1::2]
    T0 = int(max(128, ((cnt_low.max() + 127) // 128) * 128))
    T1 = int(max(128, ((cnt_high.max() + 127) // 128) * 128))

    pos_in_group = np.arange(len(order)) - starts[key_s]
    slot = pos_in_group + np.where(key_s % 2 == 1, T0, 0)

    tg_s = key_s // 2
    core_s = tg_s // TILES
    tloc_s = tg_s % TILES
    rel_s = rel[order]
    val_s = val[order]
    c2_s = c2[order]
    idxval_s = np.where(key_s % 2 == 1, c2_s - SPLIT, c2_s).astype(np.int16)

    dtnp = np.float16 if FP16 else np.float32
    T = T0 + T1
    NCH = T // 128
    sp_arr = np.zeros((NCORES, TILES, 128, NCH * 128), dtype=dtnp)
    sp_arr[core_s, tloc_s, slot % 128, (slot // 128) * 128 + rel_s] = val_s

    idx_full = np.zeros((NCORES, TILES, T), dtype=np.int16)
    idx_full[core_s, tloc_s, slot] = idxval_s
    wl = idx_full[:, :, :T0].reshape(NCORES, TILES, T0 // 16, 16)
    wh = idx_full[:, :, T0:].reshape(NCORES, TILES, T1 // 16, 16)
    band = np.concatenate([wl.transpose(0, 1, 3, 2),
                           wh.transpose(0, 1, 3, 2)], axis=3)
    wrapped = np.tile(band, (1, 1, 8, 1))

    dis_pad = np.zeros(NPAD, dtype=np.float32)
    dis_pad[:N] = dis
    dcol = dis_pad.reshape(NCORES, TILES, 128).transpose(0, 2, 1)
    discol = np.concatenate([dcol, 2.0 * dcol], axis=2).copy()

    xt = np.ascontiguousarray(
        np.asarray(x, dtype=np.float32).transpose(1, 0, 2, 3).reshape(N, D))
    xin = np.zeros((NPAD, D), dtype=dtnp)
    xin[:N] = xt.astype(dtnp)

    return xin, sp_arr, wrapped, discol, T0, T1


def _prepare_all(x, edge_index, edge_weight, weight, bias):
    xin, sp_arr, wrapped, discol, T0, T1 = _preprocess(x, edge_index, edge_weight)

    dtnp = np.float16 if FP16 else np.float32
    wts = np.ascontiguousarray(
        np.asarray(weight, dtype=np.float32).reshape(K, H * F, G)
        .transpose(1, 0, 2).reshape(128, K * G)).astype(dtnp)
    biasrep = np.tile(np.asarray(bias, dtype=np.float32)[None, :], (128, 1))
    ident = np.eye(128, dtype=np.float32)

    key = (T0, T1, FP16, N, TILES)
    if key not in _cache:
        _cache[key] = _build_program(T0, T1)
    nc = _cache[key]

    in_maps = []
    for c in range(NCORES):
        in_maps.append({
            "xin": xin,
            "sp": sp_arr[c],
            "idx": wrapped[c],
            "discol": discol[c],
            "wts": wts,
            "biasrep": biasrep,
            "ident": ident,
            "xown": xin[c * NPC:(c + 1) * NPC],
        })
    return nc, in_maps


def kernel(x, edge_index, edge_weight, weight, bias):
    import concourse.bass_utils as bass_utils

    nc, in_maps = _prepare_all(x, edge_index, edge_weight, weight, bias)
    global LAST_EXEC_NS, LAST_RESULT
    res = bass_utils.run_bass_kernel_spmd(nc, in_maps,
                                          core_ids=list(range(NCORES)),
                                          trace=TRACE)
    LAST_EXEC_NS = res.exec_time_ns
    LAST_RESULT = res
    outs = [res.results[c]["out"] for c in range(NCORES)]
    full = np.concatenate(outs, axis=1)[:, :N, :]
    return full.astype(np.float32)

